# revision 1
# baseline (speedup 1.0000x reference)
"""Trainium2 kernel for nn_BernoulliIndependentGenerator.

Strategy (data-parallel over batch, per sharding hint):
  - Host: embedding gather (index manipulation only).
  - Device (8 NeuronCores, 2 samples/core): the FLOP-heavy input
    projections xp = emb @ [w_ih_f | w_ih_b].T as tiled fp32 matmuls.
  - Host: sequential BiLSTM scan (1024 steps), gate scores, per-row
    top-k -> binary mask. Backward direction handled by shifting each
    sample's valid prefix to the end of the buffer so an unmasked
    reverse scan reproduces packed-sequence semantics.
"""

import numpy as np

B, S, E, H, V = 16, 1024, 256, 256, 50257
FOUR_H = 4 * H          # 1024
N_CORES = 8
BPC = B // N_CORES      # samples per core = 2
TOK = BPC * S           # tokens per core = 2048
BUDGET = 10


def _build_nc():
    import concourse.bass as bass
    import concourse.mybir as mybir
    from concourse.tile import TileContext

    nc = bass.Bass("TRN2")
    # packed input: [128, 8192] = [embT_k0 | embT_k1 | w_k0 | w_k1] blocks of 2048 cols
    inp = nc.dram_tensor("inp", [128, 4 * 2048], mybir.dt.float32, kind="ExternalInput")
    out = nc.dram_tensor("out", [TOK, 2 * FOUR_H], mybir.dt.float32, kind="ExternalOutput")

    KT = E // 128          # 2 k-tiles
    MT = TOK // 128        # 16 token tiles
    NT = (2 * FOUR_H) // 512  # 4 n-tiles of 512

    with TileContext(nc) as tc:
        with (
            tc.tile_pool(name="const", bufs=1) as cpool,
            tc.tile_pool(name="psum", bufs=8, space="PSUM") as ppool,
        ):
            big = cpool.tile([128, 4 * 2048], mybir.dt.float32, tag="inp")
            nc.gpsimd.dma_start(big[:], inp[:, :])
            st_all = cpool.tile([128, MT * 2 * FOUR_H], mybir.dt.float32, tag="st")

            for m in range(MT):
                for n in range(NT):
                    ps = ppool.tile([128, 512], mybir.dt.float32)
                    for k in range(KT):
                        nc.tensor.matmul(
                            ps[:],
                            big[:, k * 2048 + m * 128:k * 2048 + (m + 1) * 128],
                            big[:, 4096 + k * 2048 + n * 512:4096 + k * 2048 + (n + 1) * 512],
                            start=(k == 0),
                            stop=(k == KT - 1),
                        )
                    nc.vector.tensor_copy(
                        st_all[:, m * 2048 + n * 512:m * 2048 + (n + 1) * 512], ps[:]
                    )
            out_v = out.rearrange("(m p) c -> p m c", p=128)      # [128, 16, 2048]
            st_v = st_all[:].rearrange("p (m c) -> p m c", c=2048)
            nc.sync.dma_start(out_v, st_v)
    return nc


_NC_CACHE = None


def _device_projections(emb):
    """emb: [B, S, E] f32 -> xp [B, S, 2*4H] f32 (fwd cols 0:1024, bwd 1024:2048).
    Falls back to numpy matmul if the device path is unavailable."""
    global _NC_CACHE
    w_cat = _device_projections._w_cat  # [E, 2*4H] f32
    import os
    import signal

    if os.environ.get("KERNEL_NO_DEVICE"):
        return (emb.reshape(B * S, E) @ w_cat).reshape(B, S, 2 * FOUR_H)

    def _alarm(signum, frame):
        raise TimeoutError("device path timed out")

    old = None
    try:
        old = signal.signal(signal.SIGALRM, _alarm)
        signal.alarm(240)
    except Exception:
        old = None
    try:
        from concourse.bass_utils import run_bass_kernel_spmd

        if _NC_CACHE is None:
            _NC_CACHE = _build_nc()
        nc = _NC_CACHE
        in_maps = []
        w_pack = np.concatenate([w_cat[0:128, :], w_cat[128:256, :]], axis=1)
        for i in range(N_CORES):
            embT_i = emb[i * BPC:(i + 1) * BPC].reshape(TOK, E).T.astype(np.float32)
            packed = np.ascontiguousarray(
                np.concatenate(
                    [embT_i[0:128, :], embT_i[128:256, :], w_pack], axis=1
                )
            )
            in_maps.append({"inp": packed})
        res = run_bass_kernel_spmd(nc, in_maps, core_ids=list(range(N_CORES)))
        xp = np.empty((B, S, 2 * FOUR_H), np.float32)
        for i in range(N_CORES):
            xp[i * BPC:(i + 1) * BPC] = res.results[i]["out"].reshape(
                BPC, S, 2 * FOUR_H
            )
        return xp
    except Exception:
        # device path unavailable: equivalent host computation
        return (emb.reshape(B * S, E) @ w_cat).reshape(B, S, 2 * FOUR_H)
    finally:
        try:
            signal.alarm(0)
            if old is not None:
                signal.signal(signal.SIGALRM, old)
        except Exception:
            pass


def _sigmoid(x):
    return 1.0 / (1.0 + np.exp(-x))


def _scan(xp, w_hh_T, reverse):
    """Unmasked LSTM scan. xp: [B, S, 4H] f32, w_hh_T: [H, 4H]. Returns h: [B, S, H]."""
    Bn, Sn, _ = xp.shape
    h = np.zeros((Bn, H), np.float32)
    c = np.zeros((Bn, H), np.float32)
    hs = np.empty((Bn, Sn, H), np.float32)
    order = range(Sn - 1, -1, -1) if reverse else range(Sn)
    for t in order:
        gates = xp[:, t, :] + h @ w_hh_T
        i = _sigmoid(gates[:, 0:H])
        f = _sigmoid(gates[:, H:2 * H])
        g = np.tanh(gates[:, 2 * H:3 * H])
        o = _sigmoid(gates[:, 3 * H:4 * H])
        c = f * c + i * g
        h = o * np.tanh(c)
        hs[:, t, :] = h
    return hs


def kernel(**inputs):
    x = np.asarray(inputs["x"]).astype(np.int64)
    mask = np.asarray(inputs["mask"]).astype(bool)
    embed_table = np.asarray(inputs["embed_table"], dtype=np.float32)
    w_ih_f = np.asarray(inputs["w_ih_f"], dtype=np.float32)
    w_hh_f = np.asarray(inputs["w_hh_f"], dtype=np.float32)
    b_f = np.asarray(inputs["b_f"], dtype=np.float32)
    w_ih_b = np.asarray(inputs["w_ih_b"], dtype=np.float32)
    w_hh_b = np.asarray(inputs["w_hh_b"], dtype=np.float32)
    b_b = np.asarray(inputs["b_b"], dtype=np.float32)
    z_w = np.asarray(inputs["z_w"], dtype=np.float32)
    z_b = np.float32(np.asarray(inputs["z_b"]))

    lengths = mask.sum(1).astype(np.int64)            # [B]

    # ---- device: input projections for both directions ----
    _device_projections._w_cat = np.ascontiguousarray(
        np.concatenate([w_ih_f.T, w_ih_b.T], axis=1)
    ).astype(np.float32)                               # [E, 2048]
    emb = embed_table[x]                               # [B, S, E]
    xp = _device_projections(emb)
    xp_f = xp[:, :, :FOUR_H] + b_f                     # [B, S, 4H]
    xp_b = xp[:, :, FOUR_H:] + b_b

    # ---- host: BiLSTM scan (packed-sequence semantics via prefix shift) ----
    h_f = _scan(xp_f, np.ascontiguousarray(w_hh_f.T), reverse=False)

    # shift each sample's valid prefix to the END, reverse-scan unmasked,
    # then shift back: h_b[b, t] = h_b_shifted[b, t + S - L_b]
    shift = (S - lengths)                              # [B]
    rows = np.arange(S)[None, :]                       # [1, S]
    src = rows - shift[:, None]                        # shifted[t] = orig[src]
    src_c = np.clip(src, 0, S - 1)
    gather_idx = src_c[:, :, None]
    xp_b_shifted = np.take_along_axis(xp_b, np.broadcast_to(gather_idx, xp_b.shape), axis=1)
    xp_b_shifted = np.where((src >= 0)[:, :, None], xp_b_shifted, 0.0).astype(np.float32)
    h_b_shifted = _scan(xp_b_shifted, np.ascontiguousarray(w_hh_b.T), reverse=True)
    dst = rows + shift[:, None]                        # h_b[t] = shifted[dst]
    dst_c = np.clip(dst, 0, S - 1)
    h_b = np.take_along_axis(
        h_b_shifted, np.broadcast_to(dst_c[:, :, None], h_b_shifted.shape), axis=1
    )
    h_b = np.where((dst < S)[:, :, None], h_b, 0.0).astype(np.float32)

    # ---- gate scores + per-row top-k ----
    scores = h_f @ z_w[:H] + h_b @ z_w[H:] + z_b       # [B, S]
    probs = _sigmoid(scores.astype(np.float32))
    probs = np.where(mask, probs, 0.0).astype(np.float32)
    k = np.round(BUDGET / 100.0 * lengths.astype(np.float32)).astype(np.int64)
    ranks = np.argsort(np.argsort(-probs, axis=1, kind="stable"), axis=1, kind="stable")
    z = ((ranks < k[:, None]) & (probs > 0)).astype(np.float32)
    z = np.where(mask, z, 0.0).astype(np.float32)
    return z



# revision 14
# speedup vs baseline: 24.1198x; 24.1198x over previous
"""Trainium2 kernel for nn_BernoulliIndependentGenerator.

Strategy (2 samples per core, both LSTM directions on-device):
  - 8 NeuronCores, each owns 2 samples and runs BOTH scan directions.
    Backward direction reads xp mirrored in time (col S-1-t); padded
    positions of the backward lanes have xp zeroed (a zero xp keeps the
    LSTM state exactly zero: i*g = sigmoid(0)*tanh(0) = 0), which
    reproduces packed-sequence semantics with no data reversal anywhere.
  - Weights for both directions (4.5 MB) are sent as 1/8-shards and
    AllGathered on-device over NeuronLink, so the host->device transfer
    is ~21 MB instead of ~50 MB (the axon tunnel is ~85 MB/s).
  - Device: input projections, the 1024-step dual-direction recurrence
    (feature-on-partition layout), per-chunk score reduction via a
    ones-vector matmul.
  - Host: embedding gather + packing, final sigmoid + per-row top-k.

The module initialises the device path at import time (bass trace, jit
compile against a persistent compilation cache, one warm-up execution),
so kernel() calls are steady-state dispatch.
"""

import os
import numpy as np

B, S, E, H, V = 16, 1024, 256, 256, 50257
FOUR_H = 4 * H            # 1024
N_CORES = 8
SPC = 2                   # sample lanes per core
CHUNK = 64
NCHUNK = S // CHUNK       # 16
BUDGET = 10
TOK = SPC * S             # 2048 token columns per core, col = 2t + s

# mb block order: mb = gate*4 + dir*2 + half  (gate order i,f,o,g so the
# sigmoid covers cols 0:24 and tanh cols 24:32 of the [128, 32] gates tile)
_GATE_OF = [0, 1, 3, 2]   # torch gate row-block (i,f,g,o) for our i,f,o,g

# full weight pack: [256, 4112] = wih_mb [0:2048] | whh_mb [2048:4096]
#                   | bias256 [4096:4104] | zw256 [4104:4112]
PACK_COLS = 4112
QCOLS = PACK_COLS // 8    # 514


def _fix_drains(nc, mybir, max_waits=1):
    """Work around walrus 'Too many sync wait commands': hoist excess sem
    waits onto preceding NoOp instructions on the same engine (engines are
    in-order at dispatch, so a preceding wait-NoOp is equivalent)."""
    for fn in nc.m.functions:
        for bb in fn.blocks:
            out = []
            for ins in bb.instructions:
                if (
                    ins.sync_info is not None
                    and len(ins.sync_info.on_wait) > max_waits
                ):
                    waits = list(ins.sync_info.on_wait)
                    for j, w in enumerate(waits[max_waits:]):
                        out.append(
                            mybir.InstNoOp(
                                name=f"{ins.name}-dwfix{j}",
                                opcode="NoOp",
                                engine=ins.engine,
                                sync_info=mybir.SyncInfo(on_wait=[w], on_update=[]),
                            )
                        )
                    ins.sync_info.on_wait = waits[:max_waits]
                out.append(ins)
            bb.instructions = out


_IN_SHAPES = dict(
    embT=(2 * 128, TOK),
    wq=(2 * 128, QCOLS),
    lpat=(128, 32),
)
_OUT_SHAPE = (NCHUNK, 8 * CHUNK)


def _build_nc():
    import concourse.bass as bass
    import concourse.mybir as mybir
    from concourse.tile import TileContext

    f32 = mybir.dt.float32
    AF = mybir.ActivationFunctionType

    nc = bass.Bass("TRN2")
    embT = nc.dram_tensor("embT", list(_IN_SHAPES["embT"]), f32, kind="ExternalInput")
    wq = nc.dram_tensor("wq", list(_IN_SHAPES["wq"]), f32, kind="ExternalInput")
    lpat = nc.dram_tensor("lpat", list(_IN_SHAPES["lpat"]), f32, kind="ExternalInput")
    scores = nc.dram_tensor("scores", list(_OUT_SHAPE), f32, kind="ExternalOutput")

    with TileContext(nc) as tc:
        with (
            tc.tile_pool(name="const", bufs=1) as cpool,
            tc.tile_pool(name="hch", bufs=2) as hpool,
            tc.tile_pool(name="gates", bufs=3) as gpool,
            tc.tile_pool(name="small", bufs=3) as tpool,
            tc.tile_pool(name="stmp", bufs=1) as smpool,
            tc.tile_pool(name="mask", bufs=1) as mpool,
            tc.tile_pool(name="dram", bufs=1, space="DRAM") as dpool,
            tc.tile_pool(name="ppsum", bufs=4, space="PSUM") as ppsum,
            tc.tile_pool(name="spsum", bufs=2, space="PSUM") as spsum,
            tc.tile_pool(name="scpsum", bufs=2, space="PSUM") as scpsum,
        ):
            # ---- weight shard AllGather over NeuronLink ----
            wq_bounce = dpool.tile([256, QCOLS], f32, tag="wqin")
            wfull = dpool.tile([8 * 256, QCOLS], f32, tag="wfull")
            nc.gpsimd.dma_start(wq_bounce[:], wq[:, :])
            nc.gpsimd.collective_compute(
                "AllGather",
                mybir.AluOpType.bypass,
                replica_groups=[list(range(N_CORES))],
                ins=[wq_bounce.opt()],
                outs=[wfull.opt()],
            )

            embT_sb = cpool.tile([128, 2 * TOK], f32, tag="embt")
            wih_sb = cpool.tile([128, 2 * 2048], f32, tag="wih")
            whh_sb = cpool.tile([128, 2 * 2048], f32, tag="whh")
            bias_sb = cpool.tile([128, 16], f32, tag="bias")
            zw_sb = cpool.tile([128, 8 * CHUNK], f32, tag="zw")
            ones_sb = cpool.tile([128, 1], f32, tag="ones")
            h0_sb = cpool.tile([128, 8], f32, tag="h0")
            c_sb = cpool.tile([128, 8], f32, tag="cstate")
            xp_sb = cpool.tile([128, 32 * S], f32, tag="xp")

            for k in (0, 1):
                nc.sync.dma_start(
                    embT_sb[:, k * TOK:(k + 1) * TOK], embT[k * 128:(k + 1) * 128, :]
                )
            # assemble wih/whh/bias/zw from the gathered quarters: quarter q
            # holds pack cols [QCOLS*q, QCOLS*(q+1)), rows 256 (2 k-tiles).
            # bias pack cols 4096:4104: rows 0:128 = bias cols 0:8, rows
            # 128:256 = bias cols 8:16. zw rows 0:128 only.
            regions = [(0, 2048, "wih"), (2048, 4096, "whh"),
                       (4096, 4104, "bias"), (4104, 4112, "zw")]
            for q in range(8):
                q0, q1 = QCOLS * q, QCOLS * (q + 1)
                for r0, r1, name in regions:
                    lo, hi = max(q0, r0), min(q1, r1)
                    if lo >= hi:
                        continue
                    for k in (0, 1):
                        src = wfull[256 * q + 128 * k:256 * q + 128 * (k + 1),
                                    lo - q0:hi - q0]
                        off = lo - r0
                        if name == "wih":
                            dst = wih_sb[:, k * 2048 + off:k * 2048 + off + hi - lo]
                        elif name == "whh":
                            dst = whh_sb[:, k * 2048 + off:k * 2048 + off + hi - lo]
                        elif name == "bias":
                            dst = bias_sb[:, 8 * k + off:8 * k + off + hi - lo]
                        else:
                            if k == 1:
                                continue
                            dst = zw_sb[:, off:off + hi - lo]
                        nc.sync.dma_start(dst, src)

            nc.vector.memset(ones_sb[:], 1.0)
            nc.vector.memset(h0_sb[:], 0.0)
            nc.vector.memset(c_sb[:], 0.0)

            # tile zw [128,8] -> [128,512]
            w = 8
            while w < 8 * CHUNK:
                nc.vector.tensor_copy(zw_sb[:, w:2 * w], zw_sb[:, 0:w])
                w *= 2
            # lpat [128,32] -> [128,512] (16 step-blocks), slot reuse after
            # the projections release embT
            lrep_sb = mpool.tile([128, 512], f32, tag="lrep")
            nc.sync.dma_start(lrep_sb[:, 0:32], lpat[:, :])
            w = 32
            while w < 512:
                nc.vector.tensor_copy(lrep_sb[:, w:2 * w], lrep_sb[:, 0:w])
                w *= 2

            # ---- input projections: xp col = 32t + 2mb + s ----
            xp_v = xp_sb[:].rearrange("p (t q) -> p t q", q=32)
            for c in range(4):  # 4 chunks of 512 token columns
                for mb in range(16):
                    ps = ppsum.tile([128, 512], f32, tag="pp")
                    for e in (0, 1):
                        nc.tensor.matmul(
                            ps[:],
                            wih_sb[:, e * 2048 + mb * 128:e * 2048 + (mb + 1) * 128],
                            embT_sb[:, e * TOK + c * 512:e * TOK + (c + 1) * 512],
                            start=(e == 0),
                            stop=(e == 1),
                        )
                    nc.scalar.activation(
                        xp_v[:, c * 256:(c + 1) * 256, 2 * mb:2 * mb + 2],
                        ps[:].rearrange("p (t s) -> p t s", s=2),
                        AF.Identity,
                        bias=bias_sb[:, mb:mb + 1],
                    )

            # ---- mask padded positions of backward lanes (and none of the
            # forward lanes: their lpat entries are S) ----
            for cc in range(64):  # 512-col chunks = 16 steps each
                tp = mpool.tile([128, 512], f32, tag="tp")
                nc.gpsimd.iota(
                    tp[:], [[1, 16], [0, 32]], base=16 * cc,
                    channel_multiplier=0,
                    allow_small_or_imprecise_dtypes=True,
                )
                mk = mpool.tile([128, 512], f32, tag="mk")
                nc.vector.tensor_tensor(
                    mk[:], tp[:], lrep_sb[:], op=mybir.AluOpType.is_lt
                )
                nc.vector.tensor_mul(
                    xp_sb[:, 512 * cc:512 * (cc + 1)],
                    xp_sb[:, 512 * cc:512 * (cc + 1)],
                    mk[:],
                )

            # ---- dual-direction LSTM scan ----
            # gates [128, 32]: col = 2*mb + s, mb = gate*4 + dir*2 + half
            # h/c   [128, 8]:  col = 4*dir + 2*half + s
            xp4 = xp_sb[:].rearrange("p (t g d q) -> p t g d q", g=4, d=2, q=4)
            hprev = h0_sb[:]
            for c in range(NCHUNK):
                hch = hpool.tile([128, 8 * CHUNK], f32, tag="hch")
                for j in range(CHUNK):
                    t = c * CHUNK + j
                    ps = spsum.tile([128, 32], f32, tag="sp")
                    for mb in range(16):
                        d = (mb >> 1) & 1
                        for kc in (0, 1):
                            nc.tensor.matmul(
                                ps[:, 2 * mb:2 * mb + 2],
                                whh_sb[:, kc * 2048 + mb * 128:kc * 2048 + (mb + 1) * 128],
                                hprev[:, 4 * d + 2 * kc:4 * d + 2 * kc + 2],
                                start=(mb == 0 and kc == 0),
                                stop=(mb == 15 and kc == 1),
                                skip_group_check=True,
                            )
                    g = gpool.tile([128, 32], f32, tag="g")
                    g4 = g[:].rearrange("p (g d q) -> p g d q", g=4, d=2)
                    ps4 = ps[:].rearrange("p (g d q) -> p g d q", g=4, d=2)
                    nc.vector.tensor_add(
                        g4[:, :, 0, :], ps4[:, :, 0, :], xp4[:, t, :, 0, :]
                    )
                    nc.vector.tensor_add(
                        g4[:, :, 1, :], ps4[:, :, 1, :], xp4[:, S - 1 - t, :, 1, :]
                    )
                    a = gpool.tile([128, 32], f32, tag="a")
                    nc.scalar.activation(a[:, 0:24], g[:, 0:24], AF.Sigmoid)
                    nc.scalar.activation(a[:, 24:32], g[:, 24:32], AF.Tanh)
                    ig = tpool.tile([128, 8], f32, tag="ig")
                    nc.vector.tensor_mul(ig[:], a[:, 0:8], a[:, 24:32])
                    nc.vector.tensor_mul(c_sb[:], c_sb[:], a[:, 8:16])
                    nc.vector.tensor_add(c_sb[:], c_sb[:], ig[:])
                    th = tpool.tile([128, 8], f32, tag="th")
                    nc.scalar.activation(th[:], c_sb[:], AF.Tanh)
                    nc.vector.tensor_mul(hch[:, 8 * j:8 * j + 8], a[:, 16:24], th[:])
                    hprev = hch[:, 8 * j:8 * j + 8]
                # chunk score: sum over partitions of h * zw
                tmp = smpool.tile([128, 8 * CHUNK], f32, tag="stmp")
                nc.vector.tensor_mul(tmp[:], hch[:], zw_sb[:])
                sps = scpsum.tile([1, 8 * CHUNK], f32, tag="scp")
                nc.tensor.matmul(sps[:], ones_sb[:, 0:1], tmp[:], start=True, stop=True)
                ssb = smpool.tile([1, 8 * CHUNK], f32, tag="ssb")
                nc.vector.tensor_copy(ssb[:], sps[:])
                nc.sync.dma_start(scores[c:c + 1, :], ssb[:])
    return nc


class _DeviceRunner:
    """Builds the bass program once and holds a reusable jitted callable."""

    def __init__(self):
        import jax
        from jax.sharding import Mesh, PartitionSpec
        from jax.experimental.shard_map import shard_map
        import concourse.mybir as mybir
        from concourse import bass2jax

        if jax.config.jax_compilation_cache_dir is None:
            jax.config.update("jax_compilation_cache_dir", "/tmp/jax_kernel_cache")
            jax.config.update("jax_persistent_cache_min_compile_time_secs", 0.0)
            jax.config.update("jax_persistent_cache_min_entry_size_bytes", -1)

        self.jax = jax
        nc = _build_nc()
        _fix_drains(nc, mybir)

        bass2jax.install_neuronx_cc_hook()
        partition_name = (
            nc.partition_id_tensor.name if nc.partition_id_tensor else None
        )
        in_names, out_names, out_avals = [], [], []
        for alloc in nc.m.functions[0].allocations:
            if not isinstance(alloc, mybir.MemoryLocationSet):
                continue
            name = alloc.memorylocations[0].name
            if alloc.kind == "ExternalInput":
                if name != partition_name:
                    in_names.append(name)
            elif alloc.kind == "ExternalOutput":
                out_names.append(name)
                out_avals.append(
                    jax.core.ShapedArray(
                        tuple(alloc.tensor_shape), mybir.dt.np(alloc.dtype)
                    )
                )
        self.in_names = in_names
        n_in, n_out = len(in_names), len(out_names)
        all_names = in_names + out_names
        if partition_name:
            all_names = all_names + [partition_name]

        def _body(*args):
            ops = list(args)
            if partition_name:
                ops.append(bass2jax.partition_id_tensor())
            outs = bass2jax._bass_exec_p.bind(
                *ops,
                out_avals=tuple(out_avals),
                in_names=tuple(all_names),
                out_names=tuple(out_names),
                lowering_input_output_aliases=(),
                sim_require_finite=True,
                sim_require_nnan=True,
                nc=nc,
            )
            return tuple(outs)

        devices = jax.devices()[:N_CORES]
        mesh = Mesh(np.asarray(devices), ("core",))
        in_specs = (PartitionSpec("core"),) * (n_in + n_out)
        out_specs = (PartitionSpec("core"),) * n_out
        self._fn = jax.jit(
            shard_map(
                _body, mesh=mesh, in_specs=in_specs, out_specs=out_specs,
                check_rep=False,
            ),
            donate_argnums=tuple(range(n_in, n_in + n_out)),
            keep_unused=True,
        )
        # warm up: compile (persistent-cache hit after first ever run) and
        # load the NEFF onto the devices.
        args = [
            np.zeros((N_CORES * _IN_SHAPES[n][0], _IN_SHAPES[n][1]), np.float32)
            for n in in_names
        ]
        out = self._fn(*args, self._zero_out())
        np.asarray(out[0])

    def _zero_out(self):
        return np.zeros((N_CORES * _OUT_SHAPE[0], _OUT_SHAPE[1]), np.float32)

    def run(self, in_maps):
        args = [
            np.concatenate([in_maps[c][n] for c in range(N_CORES)], axis=0)
            for n in self.in_names
        ]
        out = self._fn(*args, self._zero_out())
        res = np.asarray(out[0]).reshape(N_CORES, *_OUT_SHAPE)
        return [res[c] for c in range(N_CORES)]


_RUNNER = None
_RUNNER_ERR = None


def _get_runner():
    global _RUNNER, _RUNNER_ERR
    if _RUNNER is None and _RUNNER_ERR is None:
        try:
            _RUNNER = _DeviceRunner()
        except Exception as e:  # pragma: no cover - fallback path
            _RUNNER_ERR = e
    return _RUNNER


def _pack_weights(w_ih_f, w_hh_f, b_f, w_ih_b, w_hh_b, b_b, z_w):
    """Full both-direction weight pack [256, 4112] in mb-block order."""
    pack = np.zeros((256, PACK_COLS), np.float32)
    wih = (w_ih_f, w_ih_b)
    whh = (w_hh_f, w_hh_b)
    bs = (b_f, b_b)
    bias16 = np.zeros((128, 16), np.float32)
    for mb in range(16):
        gate = mb >> 2
        d = (mb >> 1) & 1
        half = mb & 1
        rows = slice(_GATE_OF[gate] * 256 + half * 128,
                     _GATE_OF[gate] * 256 + half * 128 + 128)
        pack[:, mb * 128:(mb + 1) * 128] = wih[d][rows].T
        pack[:, 2048 + mb * 128:2048 + (mb + 1) * 128] = whh[d][rows].T
        bias16[:, mb] = bs[d][rows]
    # zw base [128, 8]: col = 4d + 2k + s -> z_w[d*256 + k*128 + p]
    zw8 = np.zeros((128, 8), np.float32)
    for d in (0, 1):
        for k in (0, 1):
            for s in (0, 1):
                zw8[:, 4 * d + 2 * k + s] = z_w[d * 256 + k * 128:
                                                d * 256 + k * 128 + 128]
    # place bias (16 cols) in cols 4096:4104 rows 0:128 (k=0) and rows
    # 128:256 (k=1 slot unused) -- but we only have 8 cols; put 16 bias
    # cols as two row-halves: rows 0:128 cols 4096:4104 = bias16[:, 0:8],
    # rows 128:256 cols 4096:4104 = bias16[:, 8:16]
    pack[0:128, 4096:4104] = bias16[:, 0:8]
    pack[128:256, 4096:4104] = bias16[:, 8:16]
    pack[0:128, 4104:4112] = zw8
    pack[128:256, 4104:4112] = 0.0
    return pack


def _sigmoid(x):
    return 1.0 / (1.0 + np.exp(-x))


# ---------------- host fallback (numpy) ----------------

def _scan_np(xp, w_hh_T, reverse):
    Bn, Sn, _ = xp.shape
    h = np.zeros((Bn, H), np.float32)
    c = np.zeros((Bn, H), np.float32)
    hs = np.empty((Bn, Sn, H), np.float32)
    order = range(Sn - 1, -1, -1) if reverse else range(Sn)
    for t in order:
        gates = xp[:, t, :] + h @ w_hh_T
        i = _sigmoid(gates[:, 0:H])
        f = _sigmoid(gates[:, H:2 * H])
        gg = np.tanh(gates[:, 2 * H:3 * H])
        o = _sigmoid(gates[:, 3 * H:4 * H])
        c = f * c + i * gg
        h = o * np.tanh(c)
        hs[:, t, :] = h
    return hs


def _host_scores(x, mask, embed_table, w_ih_f, w_hh_f, b_f, w_ih_b, w_hh_b,
                 b_b, z_w, lengths):
    emb = embed_table[x]
    w_cat = np.concatenate([w_ih_f.T, w_ih_b.T], axis=1).astype(np.float32)
    xp = (emb.reshape(B * S, E) @ w_cat).reshape(B, S, 2 * FOUR_H)
    xp_f = xp[:, :, :FOUR_H] + b_f
    xp_b = xp[:, :, FOUR_H:] + b_b
    h_f = _scan_np(xp_f, np.ascontiguousarray(w_hh_f.T), reverse=False)
    shift = S - lengths
    rows = np.arange(S)[None, :]
    src = rows - shift[:, None]
    src_c = np.clip(src, 0, S - 1)
    xp_b_sh = np.take_along_axis(
        xp_b, np.broadcast_to(src_c[:, :, None], xp_b.shape), axis=1
    )
    xp_b_sh = np.where((src >= 0)[:, :, None], xp_b_sh, 0.0).astype(np.float32)
    h_b_sh = _scan_np(xp_b_sh, np.ascontiguousarray(w_hh_b.T), reverse=True)
    dst = rows + shift[:, None]
    dst_c = np.clip(dst, 0, S - 1)
    h_b = np.take_along_axis(
        h_b_sh, np.broadcast_to(dst_c[:, :, None], h_b_sh.shape), axis=1
    )
    h_b = np.where((dst < S)[:, :, None], h_b, 0.0).astype(np.float32)
    return h_f @ z_w[:H] + h_b @ z_w[H:]


def _device_scores(emb, lengths, pack):
    runner = _get_runner()
    if runner is None:
        raise RuntimeError(f"device unavailable: {_RUNNER_ERR!r}")
    in_maps = []
    for core in range(N_CORES):
        blk = emb[SPC * core:SPC * core + SPC]          # [2, S, E]
        embT = np.ascontiguousarray(
            blk.transpose(2, 1, 0).reshape(E, TOK)
        ).astype(np.float32)
        wq = np.ascontiguousarray(pack[:, QCOLS * core:QCOLS * (core + 1)])
        lp = np.zeros((128, 32), np.float32)
        for mb in range(16):
            d = (mb >> 1) & 1
            for s in range(SPC):
                lp[:, 2 * mb + s] = S if d == 0 else lengths[SPC * core + s]
        in_maps.append(dict(embT=embT, wq=wq, lpat=lp))
    results = runner.run(in_maps)

    s_f = np.empty((B, S), np.float32)
    s_b = np.empty((B, S), np.float32)
    for core in range(N_CORES):
        r = np.asarray(results[core], dtype=np.float32)
        # col = 8j + 4d + 2k + s; sum over k
        sc = r.reshape(NCHUNK * CHUNK, 2, 2, 2).sum(axis=2)   # [j, d, s]
        for s in range(SPC):
            s_f[SPC * core + s] = sc[:, 0, s]
            s_b[SPC * core + s] = sc[::-1, 1, s]              # j = S-1-t
    return s_f, s_b


def kernel(**inputs):
    x = np.asarray(inputs["x"]).astype(np.int64)
    mask = np.asarray(inputs["mask"]).astype(bool)
    embed_table = np.asarray(inputs["embed_table"], dtype=np.float32)
    w_ih_f = np.asarray(inputs["w_ih_f"], dtype=np.float32)
    w_hh_f = np.asarray(inputs["w_hh_f"], dtype=np.float32)
    b_f = np.asarray(inputs["b_f"], dtype=np.float32)
    w_ih_b = np.asarray(inputs["w_ih_b"], dtype=np.float32)
    w_hh_b = np.asarray(inputs["w_hh_b"], dtype=np.float32)
    b_b = np.asarray(inputs["b_b"], dtype=np.float32)
    z_w = np.asarray(inputs["z_w"], dtype=np.float32)
    z_b = np.float32(np.asarray(inputs["z_b"]))

    lengths = mask.sum(1).astype(np.int64)

    try:
        if os.environ.get("KERNEL_NO_DEVICE"):
            raise RuntimeError("forced host path")
        import signal

        def _alarm(signum, frame):
            raise TimeoutError("device path timed out")

        old = None
        try:
            old = signal.signal(signal.SIGALRM, _alarm)
            signal.alarm(240)
        except Exception:
            old = None
        try:
            emb = embed_table[x]                              # [B, S, E]
            pack = _pack_weights(w_ih_f, w_hh_f, b_f, w_ih_b, w_hh_b, b_b, z_w)
            s_f, s_b = _device_scores(emb, lengths, pack)
            scores = s_f + s_b + z_b
        finally:
            try:
                signal.alarm(0)
                if old is not None:
                    signal.signal(signal.SIGALRM, old)
            except Exception:
                pass
    except Exception:
        scores = (
            _host_scores(x, mask, embed_table, w_ih_f, w_hh_f, b_f, w_ih_b,
                         w_hh_b, b_b, z_w, lengths) + z_b
        )

    probs = _sigmoid(scores.astype(np.float32))
    probs = np.where(mask, probs, 0.0).astype(np.float32)
    k = np.round(BUDGET / 100.0 * lengths.astype(np.float32)).astype(np.int64)
    ranks = np.argsort(np.argsort(-probs, axis=1, kind="stable"), axis=1, kind="stable")
    z = ((ranks < k[:, None]) & (probs > 0)).astype(np.float32)
    z = np.where(mask, z, 0.0).astype(np.float32)
    return z


# Initialise the device path at import time so the first kernel() call is
# steady-state dispatch (trace + compile + NEFF load happen here).
if not os.environ.get("KERNEL_NO_DEVICE"):
    _get_runner()


# revision 27
# speedup vs baseline: 26.4055x; 1.0948x over previous
"""Trainium2 kernel for nn_BernoulliIndependentGenerator.

Strategy (2 samples per core, both LSTM directions on-device):
  - 8 NeuronCores, each owns 2 samples and runs BOTH scan directions.
    Backward direction reads xp mirrored in time (col S-1-t); padded
    positions of the backward lanes have xp zeroed (a zero xp keeps the
    LSTM state exactly zero: i*g = sigmoid(0)*tanh(0) = 0), which
    reproduces packed-sequence semantics with no data reversal anywhere.
  - Weights for both directions (4.5 MB) are sent as 1/8-shards and
    AllGathered on-device over NeuronLink, so the host->device transfer
    is ~21 MB instead of ~50 MB (the axon tunnel is ~85 MB/s).
  - Device: input projections, the 1024-step dual-direction recurrence
    (feature-on-partition layout), per-chunk score reduction via a
    ones-vector matmul.
  - Host: embedding gather + packing, final sigmoid + per-row top-k.

The module initialises the device path at import time (bass trace, jit
compile against a persistent compilation cache, one warm-up execution),
so kernel() calls are steady-state dispatch.
"""

import os
import numpy as np

B, S, E, H, V = 16, 1024, 256, 256, 50257
FOUR_H = 4 * H            # 1024
N_CORES = 8
SPC = 2                   # sample lanes per core
CHUNK = 64
NCHUNK = S // CHUNK       # 16
BUDGET = 10
TOK = SPC * S             # 2048 token columns per core, col = 2t + s

# mb block order: mb = gate*4 + dir*2 + half  (gate order i,f,o,g so the
# sigmoid covers cols 0:24 and tanh cols 24:32 of the [128, 32] gates tile)
_GATE_OF = [0, 1, 3, 2]   # torch gate row-block (i,f,g,o) for our i,f,o,g

# full weight pack: [256, 4112] = wih_mb [0:2048] | whh_mb [2048:4096]
#                   | bias256 [4096:4104] | zw256 [4104:4112]
PACK_COLS = 4112
QCOLS = PACK_COLS // 8    # 514


def _fix_drains(nc, mybir, max_waits=1):
    """Work around walrus 'Too many sync wait commands': hoist excess sem
    waits onto preceding NoOp instructions on the same engine (engines are
    in-order at dispatch, so a preceding wait-NoOp is equivalent)."""
    for fn in nc.m.functions:
        for bb in fn.blocks:
            out = []
            for ins in bb.instructions:
                if (
                    ins.sync_info is not None
                    and len(ins.sync_info.on_wait) > max_waits
                ):
                    waits = list(ins.sync_info.on_wait)
                    for j, w in enumerate(waits[max_waits:]):
                        out.append(
                            mybir.InstNoOp(
                                name=f"{ins.name}-dwfix{j}",
                                opcode="NoOp",
                                engine=ins.engine,
                                sync_info=mybir.SyncInfo(on_wait=[w], on_update=[]),
                            )
                        )
                    ins.sync_info.on_wait = waits[:max_waits]
                out.append(ins)
            bb.instructions = out


_IN_SHAPES = dict(
    embT=(2 * 128, TOK),
    wq=(2 * 128, QCOLS),
    lpat=(128, 32),
)
_OUT_SHAPE = (NCHUNK, 8 * CHUNK)


def _build_nc():
    import concourse.bass as bass
    import concourse.mybir as mybir
    from concourse.tile import TileContext

    f32 = mybir.dt.float32
    AF = mybir.ActivationFunctionType

    nc = bass.Bass("TRN2")
    embT = nc.dram_tensor("embT", list(_IN_SHAPES["embT"]), f32, kind="ExternalInput")
    wq = nc.dram_tensor("wq", list(_IN_SHAPES["wq"]), f32, kind="ExternalInput")
    lpat = nc.dram_tensor("lpat", list(_IN_SHAPES["lpat"]), f32, kind="ExternalInput")
    scores = nc.dram_tensor("scores", list(_OUT_SHAPE), f32, kind="ExternalOutput")

    with TileContext(nc) as tc:
        with (
            tc.tile_pool(name="const", bufs=1) as cpool,
            tc.tile_pool(name="hch", bufs=2) as hpool,
            tc.tile_pool(name="gates", bufs=3) as gpool,
            tc.tile_pool(name="small", bufs=3) as tpool,
            tc.tile_pool(name="stmp", bufs=1) as smpool,
            tc.tile_pool(name="mask", bufs=1) as mpool,
            tc.tile_pool(name="dram", bufs=1, space="DRAM") as dpool,
            tc.tile_pool(name="ppsum", bufs=4, space="PSUM") as ppsum,
            tc.tile_pool(name="spsum", bufs=2, space="PSUM") as spsum,
            tc.tile_pool(name="scpsum", bufs=2, space="PSUM") as scpsum,
        ):
            # ---- weight shard AllGather over NeuronLink ----
            wq_bounce = dpool.tile([256, QCOLS], f32, tag="wqin")
            wfull = dpool.tile([8 * 256, QCOLS], f32, tag="wfull")
            nc.gpsimd.dma_start(wq_bounce[:], wq[:, :])
            nc.gpsimd.collective_compute(
                "AllGather",
                mybir.AluOpType.bypass,
                replica_groups=[list(range(N_CORES))],
                ins=[wq_bounce.opt()],
                outs=[wfull.opt()],
            )

            embT_sb = cpool.tile([128, 2 * TOK], f32, tag="embt")
            wih_sb = cpool.tile([128, 2 * 2048], f32, tag="wih")
            whh_sb = cpool.tile([128, 2 * 2048], f32, tag="whh")
            bias_sb = cpool.tile([128, 16], f32, tag="bias")
            zw_sb = cpool.tile([128, 8 * CHUNK], f32, tag="zw")
            ones_sb = cpool.tile([128, 1], f32, tag="ones")
            h0_sb = cpool.tile([128, 8], f32, tag="h0")
            c_sb = cpool.tile([128, 8], f32, tag="cstate")
            xp_sb = cpool.tile([128, 32 * S], f32, tag="xp")

            for k in (0, 1):
                nc.sync.dma_start(
                    embT_sb[:, k * TOK:(k + 1) * TOK], embT[k * 128:(k + 1) * 128, :]
                )
            # assemble wih/whh/bias/zw from the gathered quarters: quarter q
            # holds pack cols [QCOLS*q, QCOLS*(q+1)), rows 256 (2 k-tiles).
            # bias pack cols 4096:4104: rows 0:128 = bias cols 0:8, rows
            # 128:256 = bias cols 8:16. zw rows 0:128 only.
            regions = [(0, 2048, "wih"), (2048, 4096, "whh"),
                       (4096, 4104, "bias"), (4104, 4112, "zw")]
            for q in range(8):
                q0, q1 = QCOLS * q, QCOLS * (q + 1)
                for r0, r1, name in regions:
                    lo, hi = max(q0, r0), min(q1, r1)
                    if lo >= hi:
                        continue
                    for k in (0, 1):
                        src = wfull[256 * q + 128 * k:256 * q + 128 * (k + 1),
                                    lo - q0:hi - q0]
                        off = lo - r0
                        if name == "wih":
                            dst = wih_sb[:, k * 2048 + off:k * 2048 + off + hi - lo]
                        elif name == "whh":
                            dst = whh_sb[:, k * 2048 + off:k * 2048 + off + hi - lo]
                        elif name == "bias":
                            dst = bias_sb[:, 8 * k + off:8 * k + off + hi - lo]
                        else:
                            if k == 1:
                                continue
                            dst = zw_sb[:, off:off + hi - lo]
                        nc.sync.dma_start(dst, src)

            nc.vector.memset(ones_sb[:], 1.0)
            nc.vector.memset(h0_sb[:], 0.0)
            nc.vector.memset(c_sb[:], 0.0)

            # tile zw [128,8] -> [128,512]
            w = 8
            while w < 8 * CHUNK:
                nc.vector.tensor_copy(zw_sb[:, w:2 * w], zw_sb[:, 0:w])
                w *= 2
            # lpat [128,32] -> [128,512] (16 step-blocks), slot reuse after
            # the projections release embT
            lrep_sb = mpool.tile([128, 512], f32, tag="lrep")
            nc.sync.dma_start(lrep_sb[:, 0:32], lpat[:, :])
            w = 32
            while w < 512:
                nc.vector.tensor_copy(lrep_sb[:, w:2 * w], lrep_sb[:, 0:w])
                w *= 2

            # ---- input projections: xp col = 32t + 2mb + s ----
            xp_v = xp_sb[:].rearrange("p (t q) -> p t q", q=32)
            for c in range(4):  # 4 chunks of 512 token columns
                for mb in range(16):
                    ps = ppsum.tile([128, 512], f32, tag="pp")
                    for e in (0, 1):
                        nc.tensor.matmul(
                            ps[:],
                            wih_sb[:, e * 2048 + mb * 128:e * 2048 + (mb + 1) * 128],
                            embT_sb[:, e * TOK + c * 512:e * TOK + (c + 1) * 512],
                            start=(e == 0),
                            stop=(e == 1),
                        )
                    nc.scalar.activation(
                        xp_v[:, c * 256:(c + 1) * 256, 2 * mb:2 * mb + 2],
                        ps[:].rearrange("p (t s) -> p t s", s=2),
                        AF.Identity,
                        bias=bias_sb[:, mb:mb + 1],
                    )

            # ---- mask padded positions of backward lanes (and none of the
            # forward lanes: their lpat entries are S) ----
            for cc in range(64):  # 512-col chunks = 16 steps each
                tp = mpool.tile([128, 512], f32, tag="tp")
                nc.gpsimd.iota(
                    tp[:], [[1, 16], [0, 32]], base=16 * cc,
                    channel_multiplier=0,
                    allow_small_or_imprecise_dtypes=True,
                )
                mk = mpool.tile([128, 512], f32, tag="mk")
                nc.vector.tensor_tensor(
                    mk[:], tp[:], lrep_sb[:], op=mybir.AluOpType.is_lt
                )
                nc.vector.tensor_mul(
                    xp_sb[:, 512 * cc:512 * (cc + 1)],
                    xp_sb[:, 512 * cc:512 * (cc + 1)],
                    mk[:],
                )

            # ---- dual-direction LSTM scan ----
            # gates [128, 32]: col = 2*mb + s, mb = gate*4 + dir*2 + half
            # h/c   [128, 8]:  col = 4*dir + 2*half + s
            xp4 = xp_sb[:].rearrange("p (t g d q) -> p t g d q", g=4, d=2, q=4)
            hprev = h0_sb[:]
            for c in range(NCHUNK):
                hch = hpool.tile([128, 8 * CHUNK], f32, tag="hch")
                for j in range(CHUNK):
                    t = c * CHUNK + j
                    ps = spsum.tile([128, 32], f32, tag="sp")
                    for mb in range(16):
                        d = (mb >> 1) & 1
                        for kc in (0, 1):
                            nc.tensor.matmul(
                                ps[:, 2 * mb:2 * mb + 2],
                                whh_sb[:, kc * 2048 + mb * 128:kc * 2048 + (mb + 1) * 128],
                                hprev[:, 4 * d + 2 * kc:4 * d + 2 * kc + 2],
                                start=(mb == 0 and kc == 0),
                                stop=(mb == 15 and kc == 1),
                                skip_group_check=True,
                            )
                    g = gpool.tile([128, 32], f32, tag="g")
                    g4 = g[:].rearrange("p (g d q) -> p g d q", g=4, d=2)
                    ps4 = ps[:].rearrange("p (g d q) -> p g d q", g=4, d=2)
                    nc.vector.tensor_add(
                        g4[:, :, 0, :], ps4[:, :, 0, :], xp4[:, t, :, 0, :]
                    )
                    nc.vector.tensor_add(
                        g4[:, :, 1, :], ps4[:, :, 1, :], xp4[:, S - 1 - t, :, 1, :]
                    )
                    a = gpool.tile([128, 32], f32, tag="a")
                    nc.scalar.activation(a[:, 0:24], g[:, 0:24], AF.Sigmoid)
                    nc.scalar.activation(a[:, 24:32], g[:, 24:32], AF.Tanh)
                    ig = tpool.tile([128, 8], f32, tag="ig")
                    nc.vector.tensor_mul(ig[:], a[:, 0:8], a[:, 24:32])
                    nc.vector.tensor_mul(c_sb[:], c_sb[:], a[:, 8:16])
                    nc.vector.tensor_add(c_sb[:], c_sb[:], ig[:])
                    th = tpool.tile([128, 8], f32, tag="th")
                    nc.scalar.activation(th[:], c_sb[:], AF.Tanh)
                    nc.vector.tensor_mul(hch[:, 8 * j:8 * j + 8], a[:, 16:24], th[:])
                    hprev = hch[:, 8 * j:8 * j + 8]
                # chunk score: sum over partitions of h * zw
                tmp = smpool.tile([128, 8 * CHUNK], f32, tag="stmp")
                nc.vector.tensor_mul(tmp[:], hch[:], zw_sb[:])
                sps = scpsum.tile([1, 8 * CHUNK], f32, tag="scp")
                nc.tensor.matmul(sps[:], ones_sb[:, 0:1], tmp[:], start=True, stop=True)
                ssb = smpool.tile([1, 8 * CHUNK], f32, tag="ssb")
                nc.vector.tensor_copy(ssb[:], sps[:])
                nc.sync.dma_start(scores[c:c + 1, :], ssb[:])
    return nc


class _DeviceRunner:
    """Builds the bass program once and holds a reusable jitted callable."""

    def __init__(self):
        import jax
        from jax.sharding import Mesh, PartitionSpec
        from jax.experimental.shard_map import shard_map
        import concourse.mybir as mybir
        from concourse import bass2jax

        if jax.config.jax_compilation_cache_dir is None:
            jax.config.update("jax_compilation_cache_dir", "/tmp/jax_kernel_cache")
            jax.config.update("jax_persistent_cache_min_compile_time_secs", 0.0)
            jax.config.update("jax_persistent_cache_min_entry_size_bytes", -1)

        self.jax = jax
        nc = _build_nc()
        _fix_drains(nc, mybir)

        bass2jax.install_neuronx_cc_hook()
        partition_name = (
            nc.partition_id_tensor.name if nc.partition_id_tensor else None
        )
        in_names, out_names, out_avals = [], [], []
        for alloc in nc.m.functions[0].allocations:
            if not isinstance(alloc, mybir.MemoryLocationSet):
                continue
            name = alloc.memorylocations[0].name
            if alloc.kind == "ExternalInput":
                if name != partition_name:
                    in_names.append(name)
            elif alloc.kind == "ExternalOutput":
                out_names.append(name)
                out_avals.append(
                    jax.core.ShapedArray(
                        tuple(alloc.tensor_shape), mybir.dt.np(alloc.dtype)
                    )
                )
        self.in_names = in_names
        n_in, n_out = len(in_names), len(out_names)
        all_names = in_names + out_names
        if partition_name:
            all_names = all_names + [partition_name]

        def _body(*args):
            ops = list(args)
            if partition_name:
                ops.append(bass2jax.partition_id_tensor())
            outs = bass2jax._bass_exec_p.bind(
                *ops,
                out_avals=tuple(out_avals),
                in_names=tuple(all_names),
                out_names=tuple(out_names),
                lowering_input_output_aliases=(),
                sim_require_finite=True,
                sim_require_nnan=True,
                nc=nc,
            )
            return tuple(outs)

        devices = jax.devices()[:N_CORES]
        mesh = Mesh(np.asarray(devices), ("core",))
        self._mesh = mesh
        in_specs = (PartitionSpec("core"),) * (n_in + n_out)
        out_specs = (PartitionSpec("core"),) * n_out
        self._fn = jax.jit(
            shard_map(
                _body, mesh=mesh, in_specs=in_specs, out_specs=out_specs,
                check_rep=False,
            ),
            donate_argnums=tuple(range(n_in, n_in + n_out)),
            keep_unused=True,
        )
        # warm up through the exact production path (device-committed args)
        # so only one executable variant exists: compile (persistent-cache
        # hit after first ever run) and load the NEFF onto the devices.
        args = {
            n: self.put(
                np.zeros((N_CORES * _IN_SHAPES[n][0], _IN_SHAPES[n][1]),
                         np.float32)
            )
            for n in in_names
        }
        self.run(args)

    def _zero_out(self):
        return np.zeros((N_CORES * _OUT_SHAPE[0], _OUT_SHAPE[1]), np.float32)

    def run(self, args_by_name):
        """args_by_name: name -> pre-concatenated [8*rows, cols] array or
        device array. Returns stacked scores [8, 16, 512]."""
        args = [args_by_name[n] for n in self.in_names]
        out = self._fn(*args, self._zero_out())
        return np.asarray(out[0]).reshape(N_CORES, *_OUT_SHAPE)

    def put(self, arr):
        """Async transfer of a pre-concatenated argument to the mesh."""
        from jax.sharding import NamedSharding, PartitionSpec
        return self.jax.device_put(
            arr, NamedSharding(self._mesh, PartitionSpec("core"))
        )


_RUNNER = None
_RUNNER_ERR = None


def _get_runner():
    global _RUNNER, _RUNNER_ERR
    if _RUNNER is None and _RUNNER_ERR is None:
        try:
            _RUNNER = _DeviceRunner()
        except Exception as e:  # pragma: no cover - fallback path
            _RUNNER_ERR = e
    return _RUNNER


def _pack_weights(w_ih_f, w_hh_f, b_f, w_ih_b, w_hh_b, b_b, z_w):
    """Full both-direction weight pack [256, 4112] in mb-block order."""
    pack = np.zeros((256, PACK_COLS), np.float32)
    wih = (w_ih_f, w_ih_b)
    whh = (w_hh_f, w_hh_b)
    bs = (b_f, b_b)
    bias16 = np.zeros((128, 16), np.float32)
    for mb in range(16):
        gate = mb >> 2
        d = (mb >> 1) & 1
        half = mb & 1
        rows = slice(_GATE_OF[gate] * 256 + half * 128,
                     _GATE_OF[gate] * 256 + half * 128 + 128)
        pack[:, mb * 128:(mb + 1) * 128] = wih[d][rows].T
        pack[:, 2048 + mb * 128:2048 + (mb + 1) * 128] = whh[d][rows].T
        bias16[:, mb] = bs[d][rows]
    # zw base [128, 8]: col = 4d + 2k + s -> z_w[d*256 + k*128 + p]
    zw8 = np.zeros((128, 8), np.float32)
    for d in (0, 1):
        for k in (0, 1):
            for s in (0, 1):
                zw8[:, 4 * d + 2 * k + s] = z_w[d * 256 + k * 128:
                                                d * 256 + k * 128 + 128]
    # place bias (16 cols) in cols 4096:4104 rows 0:128 (k=0) and rows
    # 128:256 (k=1 slot unused) -- but we only have 8 cols; put 16 bias
    # cols as two row-halves: rows 0:128 cols 4096:4104 = bias16[:, 0:8],
    # rows 128:256 cols 4096:4104 = bias16[:, 8:16]
    pack[0:128, 4096:4104] = bias16[:, 0:8]
    pack[128:256, 4096:4104] = bias16[:, 8:16]
    pack[0:128, 4104:4112] = zw8
    pack[128:256, 4104:4112] = 0.0
    return pack


def _sigmoid(x):
    return 1.0 / (1.0 + np.exp(-x))


# ---------------- host fallback (numpy) ----------------

def _scan_np(xp, w_hh_T, reverse):
    Bn, Sn, _ = xp.shape
    h = np.zeros((Bn, H), np.float32)
    c = np.zeros((Bn, H), np.float32)
    hs = np.empty((Bn, Sn, H), np.float32)
    order = range(Sn - 1, -1, -1) if reverse else range(Sn)
    for t in order:
        gates = xp[:, t, :] + h @ w_hh_T
        i = _sigmoid(gates[:, 0:H])
        f = _sigmoid(gates[:, H:2 * H])
        gg = np.tanh(gates[:, 2 * H:3 * H])
        o = _sigmoid(gates[:, 3 * H:4 * H])
        c = f * c + i * gg
        h = o * np.tanh(c)
        hs[:, t, :] = h
    return hs


def _host_scores(x, mask, embed_table, w_ih_f, w_hh_f, b_f, w_ih_b, w_hh_b,
                 b_b, z_w, lengths):
    emb = embed_table[x]
    w_cat = np.concatenate([w_ih_f.T, w_ih_b.T], axis=1).astype(np.float32)
    xp = (emb.reshape(B * S, E) @ w_cat).reshape(B, S, 2 * FOUR_H)
    xp_f = xp[:, :, :FOUR_H] + b_f
    xp_b = xp[:, :, FOUR_H:] + b_b
    h_f = _scan_np(xp_f, np.ascontiguousarray(w_hh_f.T), reverse=False)
    shift = S - lengths
    rows = np.arange(S)[None, :]
    src = rows - shift[:, None]
    src_c = np.clip(src, 0, S - 1)
    xp_b_sh = np.take_along_axis(
        xp_b, np.broadcast_to(src_c[:, :, None], xp_b.shape), axis=1
    )
    xp_b_sh = np.where((src >= 0)[:, :, None], xp_b_sh, 0.0).astype(np.float32)
    h_b_sh = _scan_np(xp_b_sh, np.ascontiguousarray(w_hh_b.T), reverse=True)
    dst = rows + shift[:, None]
    dst_c = np.clip(dst, 0, S - 1)
    h_b = np.take_along_axis(
        h_b_sh, np.broadcast_to(dst_c[:, :, None], h_b_sh.shape), axis=1
    )
    h_b = np.where((dst < S)[:, :, None], h_b, 0.0).astype(np.float32)
    return h_f @ z_w[:H] + h_b @ z_w[H:]


def _device_scores(emb, lengths, pack):
    runner = _get_runner()
    if runner is None:
        raise RuntimeError(f"device unavailable: {_RUNNER_ERR!r}")
    # wq: [8*256, 514] with arg[256q + r, c] = pack[r, QCOLS*q + c]
    wq = np.ascontiguousarray(
        pack.T.reshape(N_CORES, QCOLS, 256).transpose(0, 2, 1).reshape(
            N_CORES * 256, QCOLS
        )
    )
    # lpat: [8*128, 32], col 2mb+s; fwd lanes S, bwd lanes length
    lp = np.empty((N_CORES, 128, 32), np.float32)
    lv = lengths.reshape(N_CORES, SPC).astype(np.float32)   # [core, s]
    for mb in range(16):
        d = (mb >> 1) & 1
        for s in range(SPC):
            lp[:, :, 2 * mb + s] = S if d == 0 else lv[:, s][:, None]
    d_wq = runner.put(wq)
    d_lp = runner.put(lp.reshape(N_CORES * 128, 32))
    # embT: [8*256, 2048] with arg[256q + e, 2t+s] = emb[2q+s, t, e]
    embT = np.ascontiguousarray(
        emb.reshape(N_CORES, SPC, S, E).transpose(0, 3, 2, 1).reshape(
            N_CORES * E, TOK
        )
    )
    d_embT = runner.put(embT)
    res = runner.run(dict(embT=d_embT, wq=d_wq, lpat=d_lp))

    # col = 8j + 4d + 2k + s; sum over k -> [core, j, d, s]
    sc = res.reshape(N_CORES, S, 2, 2, SPC).sum(axis=3)
    s_f = sc[:, :, 0, :].transpose(0, 2, 1).reshape(B, S)
    s_b = sc[:, ::-1, 1, :].transpose(0, 2, 1).reshape(B, S)  # j = S-1-t
    return s_f, s_b


def kernel(**inputs):
    x = np.asarray(inputs["x"]).astype(np.int64)
    mask = np.asarray(inputs["mask"]).astype(bool)
    embed_table = np.asarray(inputs["embed_table"], dtype=np.float32)
    w_ih_f = np.asarray(inputs["w_ih_f"], dtype=np.float32)
    w_hh_f = np.asarray(inputs["w_hh_f"], dtype=np.float32)
    b_f = np.asarray(inputs["b_f"], dtype=np.float32)
    w_ih_b = np.asarray(inputs["w_ih_b"], dtype=np.float32)
    w_hh_b = np.asarray(inputs["w_hh_b"], dtype=np.float32)
    b_b = np.asarray(inputs["b_b"], dtype=np.float32)
    z_w = np.asarray(inputs["z_w"], dtype=np.float32)
    z_b = np.float32(np.asarray(inputs["z_b"]))

    lengths = mask.sum(1).astype(np.int64)

    try:
        if os.environ.get("KERNEL_NO_DEVICE"):
            raise RuntimeError("forced host path")
        import signal

        def _alarm(signum, frame):
            raise TimeoutError("device path timed out")

        old = None
        try:
            old = signal.signal(signal.SIGALRM, _alarm)
            signal.alarm(240)
        except Exception:
            old = None
        try:
            emb = embed_table[x]                              # [B, S, E]
            pack = _pack_weights(w_ih_f, w_hh_f, b_f, w_ih_b, w_hh_b, b_b, z_w)
            s_f, s_b = _device_scores(emb, lengths, pack)
            scores = s_f + s_b + z_b
        finally:
            try:
                signal.alarm(0)
                if old is not None:
                    signal.signal(signal.SIGALRM, old)
            except Exception:
                pass
    except Exception:
        scores = (
            _host_scores(x, mask, embed_table, w_ih_f, w_hh_f, b_f, w_ih_b,
                         w_hh_b, b_b, z_w, lengths) + z_b
        )

    probs = _sigmoid(scores.astype(np.float32))
    probs = np.where(mask, probs, 0.0).astype(np.float32)
    k = np.round(BUDGET / 100.0 * lengths.astype(np.float32)).astype(np.int64)
    ranks = np.argsort(np.argsort(-probs, axis=1, kind="stable"), axis=1, kind="stable")
    z = ((ranks < k[:, None]) & (probs > 0)).astype(np.float32)
    z = np.where(mask, z, 0.0).astype(np.float32)
    return z


# Initialise the device path at import time so the first kernel() call is
# steady-state dispatch (trace + compile + NEFF load happen here). Guarded
# by an alarm so a wedged device degrades to the host fallback instead of
# hanging the caller.
if not os.environ.get("KERNEL_NO_DEVICE"):
    try:
        import signal as _signal

        def _init_alarm(signum, frame):
            raise TimeoutError("device init timed out")

        _old = _signal.signal(_signal.SIGALRM, _init_alarm)
        _signal.alarm(900)
    except Exception:
        _old = None
    try:
        _get_runner()
    except Exception as _e:
        _RUNNER_ERR = _RUNNER_ERR or _e
    finally:
        try:
            _signal.alarm(0)
            if _old is not None:
                _signal.signal(_signal.SIGALRM, _old)
        except Exception:
            pass
